# revision 17
# baseline (speedup 1.0000x reference)
"""GATv2 encoder (2-layer, PyG-style) on 8 Trainium2 NeuronCores — v2.

Strategy (graph/data parallel):
  - Nodes are assigned to 392 (=8*49) 128-node blocks by a degree-balanced
    greedy so every block owns <=1152 edge slots (9 tiles of 128 edges);
    each core owns 49 consecutive blocks.
  - Edges (incl. self-loops) are routed to the block owning the destination
    node, sorted by destination, and packed sequentially into 128-edge tiles
    (PSUM accumulation across a block's tiles handles nodes spanning tiles).
  - Layer-1 per-edge source features are fully materialized on the host,
    pre-transposed (fp16), and streamed per block — no device-side gather
    and no per-tile transposes for layer 1.
  - Layer-2 per-edge source features are fetched with per-tile indirect
    (gather) DMA from an fp16 node-feature table that is built per-core for
    owned nodes and exchanged with an fp16 AllGather collective.
  - Per-tile 0/1 selection matrices (fp16) stream from DRAM with one batched
    DMA per block; segment softmax + weighted aggregation run on the tensor
    engine with fp16 operands and fp32 PSUM accumulation.
  - leaky_relu runs natively on the scalar engine (Prelu, alpha=0.2); a
    fraction of per-tile logit work is offloaded to the GpSimd engine.

kernel(**inputs) takes the full-size inputs and returns the full [N, 256]
output; all sharding happens inside.
"""

import heapq
from contextlib import ExitStack

import numpy as np

import concourse.bass as bass
import concourse.tile as tile
from concourse import bacc, mybir
from concourse.bass_utils import run_bass_kernel_spmd

F32 = mybir.dt.float32
F16 = mybir.dt.float16
I32 = mybir.dt.int32

NEG_SLOPE = 0.2
P = 128
H, CH, C = 4, 64, 256
POOL8 = 3   # of every 8 layer-1 tile-pairs, this many run lm on GpSimd
DVEP8 = 0   # leaky-relu on DVE needs two PSUM reads (illegal); keep on ACT
SC8 = 5     # of every 8 layer-2 tiles, this many run the w*pre scale on DVE


# ---------------------------------------------------------------------------
# Host-side preprocessing
# ---------------------------------------------------------------------------

def preprocess(x, edge_index, n_cores=8):
    N = x.shape[0]
    IN1 = x.shape[1] + 1
    src0 = np.concatenate([edge_index[0], np.arange(N, dtype=np.int64)])
    dst0 = np.concatenate([edge_index[1], np.arange(N, dtype=np.int64)])
    deg = np.bincount(dst0, minlength=N)

    nblk = (N + P - 1) // P
    nbpc = (nblk + n_cores - 1) // n_cores
    nblk_pad = nbpc * n_cores
    npad = nblk_pad * P

    # degree-balanced node -> (block, slot) assignment
    order = np.argsort(-deg, kind="stable")
    binedges = np.zeros(nblk_pad, dtype=np.int64)
    binslots = np.full(nblk_pad, P, dtype=np.int64)
    newid = np.empty(N, dtype=np.int64)
    heap = [(0, b) for b in range(nblk_pad)]
    heapq.heapify(heap)
    for node in order:
        while True:
            _, b = heapq.heappop(heap)
            if binslots[b] > 0:
                break
        newid[node] = b * P + (P - binslots[b])
        binslots[b] -= 1
        binedges[b] += deg[node]
        if binslots[b] > 0:
            heapq.heappush(heap, (int(binedges[b]), b))
    tpb = int(np.ceil(binedges.max() / P))

    srcP = newid[src0]
    dstP = newid[dst0]
    order_e = np.argsort(dstP, kind="stable")
    srcO_s = src0[order_e]
    srcP_s = srcP[order_e]
    dstP_s = dstP[order_e]

    binof = dstP_s // P
    bin_start = np.searchsorted(dstP_s, np.arange(nblk_pad) * P)
    e_pos = np.arange(len(dstP_s)) - bin_start[binof]
    col_glob = binof * tpb + e_pos // P
    p_arr = e_pos % P
    r_arr = dstP_s % P
    totcols = nblk_pad * tpb

    gidx_g = np.zeros((P, totcols), dtype=np.int32)
    gidx_g[p_arr, col_glob] = srcP_s
    selD = np.zeros((totcols, P, P), dtype=np.uint8)
    selD[col_glob, p_arr, r_arr] = 1

    x_aug = np.concatenate([x.astype(np.float32),
                            np.ones((N, 1), np.float32)], axis=1)
    xe = x_aug[srcO_s].astype(np.float16)           # [Etot, IN1]
    xeT_g = np.zeros((IN1, totcols * P), dtype=np.float16)
    xeT_g[:, col_glob * P + p_arr] = xe.T

    xT_perm = np.zeros((IN1, npad), dtype=np.float16)
    xT_perm[:, newid] = x_aug.T.astype(np.float16)

    cfg = dict(N=N, IN1=IN1, npad=npad, nblk=nblk_pad, nbpc=nbpc, tpb=tpb,
               n_cores=n_cores)
    per_core = []
    cpc = nbpc * tpb
    for c in range(n_cores):
        sl = slice(c * cpc, (c + 1) * cpc)
        blk = selD[sl]
        per_core.append(dict(
            sel=np.ascontiguousarray(
                blk.transpose(1, 0, 2).reshape(P, cpc * P)).astype(np.float16),
            selt=np.ascontiguousarray(
                blk.transpose(2, 0, 1).reshape(P, cpc * P)).astype(np.float16),
            gidx=np.ascontiguousarray(gidx_g[:, sl]),
            xeT=np.ascontiguousarray(xeT_g[:, c * cpc * P:(c + 1) * cpc * P]),
            xT=np.ascontiguousarray(
                xT_perm[:, c * nbpc * P:(c + 1) * nbpc * P]),
        ))
    return cfg, per_core, newid


def pack_weights(inputs, cfg):
    IN1 = cfg["IN1"]
    f16 = np.float16
    w1 = np.zeros((IN1, 512), dtype=f16)
    w1[:IN1 - 1, 0:256] = np.asarray(inputs["W_l1"], np.float32).astype(f16)
    w1[IN1 - 1, 0:256] = np.asarray(inputs["b_l1"], np.float32).astype(f16)
    w1[:IN1 - 1, 256:512] = np.asarray(inputs["W_r1"], np.float32).astype(f16)
    w1[IN1 - 1, 256:512] = np.asarray(inputs["b_r1"], np.float32).astype(f16)

    W_l2 = np.asarray(inputs["W_l2"], np.float32)
    W_r2 = np.asarray(inputs["W_r2"], np.float32)
    w2 = np.concatenate([W_l2, W_r2], axis=1)            # [256, 512]
    w2_sb = np.zeros((P, 1024), dtype=f16)
    w2_sb[:, 0:512] = w2[0:P].astype(f16)
    w2_sb[:, 512:1024] = w2[P:2 * P].astype(f16)
    w2bias = np.concatenate([
        np.asarray(inputs["b_l2"], np.float32) - W_l2.sum(axis=0),
        np.asarray(inputs["b_r2"], np.float32) - W_r2.sum(axis=0),
    ])[None, :].astype(f16)                               # [1, 512]

    att1 = np.asarray(inputs["att1"], np.float32).reshape(-1)
    att2 = np.asarray(inputs["att2"], np.float32).reshape(-1)
    return dict(
        w1=w1, w2=w2_sb, w2bias=w2bias,
        att1m=np.broadcast_to(np.concatenate([att1, att1]),
                              (P, 512)).astype(f16),
        att2m=np.broadcast_to(att2, (P, 256)).astype(f16),
        b1m=np.broadcast_to(np.asarray(inputs["bias1"], np.float32),
                            (P, 256)).astype(f16),
        b2m=np.broadcast_to(np.asarray(inputs["bias2"], np.float32),
                            (P, 256)).astype(f16),
        ones=np.ones((1, P), dtype=f16),
        ident=np.eye(P, dtype=f16),
    )


# ---------------------------------------------------------------------------
# Device program
# ---------------------------------------------------------------------------

def bcast_heads(ap, nh, chw):
    # [P, nh] -> [P, nh, chw] with stride-0 inner broadcast
    return bass.AP(ap.tensor, ap.offset, [ap.ap[0], [1, nh], [0, chw]])


def bcast_cols(ap, n):
    # [P, 1] -> [P, n] with stride-0 free broadcast
    return bass.AP(ap.tensor, ap.offset, [ap.ap[0], [0, n]])


def build_program(cfg):
    npad, nbpc, tpb, n_cores = cfg["npad"], cfg["nbpc"], cfg["tpb"], cfg["n_cores"]
    IN1 = cfg["IN1"]
    nown = nbpc * P
    cpc = nbpc * tpb
    TW = tpb * P

    nc = bacc.Bacc("TRN2", target_bir_lowering=False, debug=False,
                   num_devices=n_cores)

    xT_d = nc.dram_tensor("xT", [IN1, nown], F16, kind="ExternalInput").ap()
    w1_d = nc.dram_tensor("w1", [IN1, 512], F16, kind="ExternalInput").ap()
    w2_d = nc.dram_tensor("w2", [P, 1024], F16, kind="ExternalInput").ap()
    w2b_d = nc.dram_tensor("w2bias", [1, 512], F16, kind="ExternalInput").ap()
    att1m_d = nc.dram_tensor("att1m", [P, 2 * C], F16, kind="ExternalInput").ap()
    att2m_d = nc.dram_tensor("att2m", [P, C], F16, kind="ExternalInput").ap()
    b1m_d = nc.dram_tensor("b1m", [P, C], F16, kind="ExternalInput").ap()
    b2m_d = nc.dram_tensor("b2m", [P, C], F16, kind="ExternalInput").ap()
    gidx_d = nc.dram_tensor("gidx", [P, cpc], I32, kind="ExternalInput").ap()
    sel_d = nc.dram_tensor("sel", [P, cpc * P], F16, kind="ExternalInput").ap()
    selt_d = nc.dram_tensor("selt", [P, cpc * P], F16, kind="ExternalInput").ap()
    xeT_d = nc.dram_tensor("xeT", [IN1, cpc * P], F16, kind="ExternalInput").ap()
    ones_d = nc.dram_tensor("ones", [1, P], F16, kind="ExternalInput").ap()
    ident_d = nc.dram_tensor("ident", [P, P], F16, kind="ExternalInput").ap()
    out_d = nc.dram_tensor("out", [nown, C], F32, kind="ExternalOutput").ap()

    xl2own_d = nc.dram_tensor("xl2own", [nown, C], F16).ap()
    xl2t_d = nc.dram_tensor("xl2t", [npad, C], F16, addr_space="Shared").ap()

    with tile.TileContext(nc) as tc, ExitStack() as ctx:
        persist = ctx.enter_context(tc.tile_pool(name="persist", bufs=1))
        blk = ctx.enter_context(tc.tile_pool(name="blk", bufs=3))
        work = ctx.enter_context(tc.tile_pool(name="work", bufs=8))
        gwork = ctx.enter_context(tc.tile_pool(name="gwork", bufs=3))
        l2g = ctx.enter_context(tc.tile_pool(name="l2g", bufs=6))
        small = ctx.enter_context(tc.tile_pool(name="small", bufs=6))
        psum = ctx.enter_context(tc.tile_pool(name="psum", bufs=3, space="PSUM"))
        psacc = ctx.enter_context(tc.tile_pool(name="psacc", bufs=2, space="PSUM"))
        psc = ctx.enter_context(tc.tile_pool(name="psc", bufs=1, space="PSUM"))

        xr_sb = persist.tile([P, nbpc * C], F16, tag="xr")
        gidx = persist.tile([P, cpc], I32, tag="gidx")
        w1sb = persist.tile([IN1, 512], F16, tag="w1sb")
        w2sb = persist.tile([P, 1024], F16, tag="w2sb")
        w2bsb = persist.tile([1, 512], F16, tag="w2bsb")
        att1m = persist.tile([P, 2 * C], F16, tag="att1m")
        att2m = persist.tile([P, C], F16, tag="att2m")
        b1m = persist.tile([P, C], F16, tag="b1m")
        b2m = persist.tile([P, C], F16, tag="b2m")
        ones1 = persist.tile([1, P], F16, tag="ones1")
        ident = persist.tile([P, P], F16, tag="ident")

        nc.sync.dma_start(gidx[:], gidx_d[:])
        nc.sync.dma_start(w1sb[:], w1_d[:])
        nc.sync.dma_start(w2sb[:], w2_d[:])
        nc.sync.dma_start(w2bsb[:], w2b_d[:])
        nc.sync.dma_start(att1m[:], att1m_d[:])
        nc.sync.dma_start(att2m[:], att2m_d[:])
        nc.sync.dma_start(b1m[:], b1m_d[:])
        nc.sync.dma_start(b2m[:], b2m_d[:])
        nc.sync.dma_start(ones1[:], ones_d[:])
        nc.sync.dma_start(ident[:], ident_d[:])

        AF = mybir.ActivationFunctionType
        OP = mybir.AluOpType

        # ---- layer 1 + node transforms, per block -------------------------
        tcnt = 0
        for j in range(nbpc):
            # x_r1 for this block
            xto = blk.tile([IN1, P], F16, tag="xto")
            nc.sync.dma_start(xto[:], xT_d[:, j * P:(j + 1) * P])
            pxr = psc.tile([P, C], F32, space="PSUM", tag="pxr")
            nc.tensor.matmul(pxr[:], xto[:], w1sb[:, C:2 * C],
                             start=True, stop=True)
            xrj = xr_sb[:, j * C:(j + 1) * C]
            nc.scalar.activation(xrj, pxr[:], AF.Copy)

            selB = blk.tile([P, TW], F16, tag="selB")
            seltB = blk.tile([P, TW], F16, tag="seltB")
            xeTB = blk.tile([IN1, TW], F16, tag="xeTB")
            nc.sync.dma_start(selB[:], sel_d[:, j * TW:(j + 1) * TW])
            nc.sync.dma_start(seltB[:], selt_d[:, j * TW:(j + 1) * TW])
            nc.sync.dma_start(xeTB[:], xeT_d[:, j * TW:(j + 1) * TW])

            po = psacc.tile([P, H + C], F32, space="PSUM", tag="po")
            for t0 in range(0, tpb, 2):
                wid = 2 if t0 + 1 < tpb else 1
                WC = wid * C
                ppx = psum.tile([P, 2 * C], F32, space="PSUM", tag="ppx")
                for u in range(wid):
                    t = t0 + u
                    sl = slice(u * C, (u + 1) * C)
                    nc.tensor.matmul(ppx[:, sl], seltB[:, t * P:(t + 1) * P],
                                     xrj, start=True, stop=False)
                    nc.tensor.matmul(ppx[:, sl], xeTB[:, t * P:(t + 1) * P],
                                     w1sb[:, 0:C], start=False, stop=True)
                lv = work.tile([P, 2 * C], F16, tag="lv")
                nc.scalar.activation(lv[:, 0:WC], ppx[:, 0:WC], AF.Prelu,
                                     alpha=NEG_SLOPE)
                eng = nc.gpsimd if (tcnt % 8) < POOL8 else nc.vector
                tcnt += 1
                lm = work.tile([P, 2 * C], F16, tag="lm")
                eng.tensor_tensor(lm[:, 0:WC], lv[:, 0:WC], att1m[:, 0:WC],
                                  op=OP.mult)
                lg = small.tile([P, 2 * H], F16, tag="lg")
                with nc.allow_low_precision(reason="fp16 attention logits"):
                    nc.vector.reduce_sum(
                        lg[:, 0:wid * H].rearrange("p (w h) -> p w h", w=wid),
                        lm[:, 0:WC].rearrange("p (w h c) -> p w h c",
                                              w=wid, h=H),
                        axis=mybir.AxisListType.X)
                wwx = work.tile([P, 2 * (H + C)], F16, tag="wwx")
                w0 = wwx[:]
                nc.scalar.activation(
                    bass.AP(w0.tensor, w0.offset,
                            [w0.ap[0], [H + C, wid], [1, H]]),
                    lg[:, 0:wid * H].rearrange("p (w h) -> p w h", w=wid),
                    AF.Exp)
                wH = wwx[:, H:]
                pp0 = ppx[:]
                nc.vector.tensor_tensor(
                    bass.AP(wH.tensor, wH.offset,
                            [wH.ap[0], [H + C, wid], [CH, H], [1, CH]]),
                    bass.AP(pp0.tensor, pp0.offset,
                            [pp0.ap[0], [C, wid], [CH, H], [1, CH]]),
                    bass.AP(w0.tensor, w0.offset,
                            [w0.ap[0], [H + C, wid], [1, H], [0, CH]]),
                    op=OP.mult)
                for u in range(wid):
                    t = t0 + u
                    nc.tensor.matmul(
                        po[:], selB[:, t * P:(t + 1) * P],
                        wwx[:, u * (H + C):(u + 1) * (H + C)],
                        start=(t == 0), stop=(t == tpb - 1))

            # block epilogue: h = agg/denom - x_r1 + bias1
            dn = small.tile([P, H], F32, tag="dn")
            nc.vector.tensor_scalar(dn[:], po[:, 0:H], 1e-30, None,
                                    op0=OP.add)
            rd = small.tile([P, H], F32, tag="rd")
            nc.vector.reciprocal(rd[:], dn[:])
            hh = gwork.tile([P, C], F16, tag="hh")
            nc.vector.tensor_tensor(
                hh[:].rearrange("p (h c) -> p h c", h=H),
                po[:, H:H + C].rearrange("p (h c) -> p h c", h=H),
                bcast_heads(rd[:], H, CH), op=OP.mult)
            nc.vector.tensor_tensor(hh[:], hh[:], xrj, op=OP.subtract)
            nc.vector.tensor_tensor(hh[:], hh[:], b1m[:], op=OP.add)

            # node transform: g = elu(h)+1; x_l2 / x_r2 = g@W - colsum(W) + b
            t1 = gwork.tile([P, C], F16, tag="t1")
            nc.vector.tensor_scalar(t1[:], hh[:], 0.0, None, op0=OP.min)
            e1 = gwork.tile([P, C], F16, tag="e1")
            nc.scalar.activation(e1[:], t1[:], AF.Exp)
            g = gwork.tile([P, C], F16, tag="g")
            nc.vector.scalar_tensor_tensor(g[:], hh[:], 0.0, e1[:],
                                           op0=OP.max, op1=OP.add)
            gTa = gwork.tile([P, P], F16, tag="gTa")
            gTb = gwork.tile([P, P], F16, tag="gTb")
            nc.sync.dma_start_transpose(gTa[:], g[:, 0:P])
            nc.sync.dma_start_transpose(gTb[:], g[:, P:2 * P])
            px = psc.tile([P, 512], F32, space="PSUM", tag="px")
            nc.tensor.matmul(px[:], gTa[:], w2sb[:, 0:512],
                             start=True, stop=False)
            nc.tensor.matmul(px[:], gTb[:], w2sb[:, 512:1024],
                             start=False, stop=False)
            nc.tensor.matmul(px[:], ones1[:], w2bsb[:],
                             start=False, stop=True)
            xs = gwork.tile([P, C], F16, tag="xs")
            nc.scalar.activation(xs[:], px[:, 0:C], AF.Copy)
            nc.sync.dma_start(xl2own_d[j * P:(j + 1) * P, :], xs[:])
            nc.scalar.activation(xrj, px[:, C:2 * C], AF.Copy)

        # ---- AllGather x_l2 ----------------------------------------------
        nc.gpsimd.collective_compute(
            "AllGather", mybir.AluOpType.bypass,
            replica_groups=[list(range(n_cores))],
            ins=[xl2own_d[:]], outs=[xl2t_d[:]])

        # ---- layer 2 edges ------------------------------------------------
        pcnt = 0
        scnt = 0
        for j in range(nbpc):
            selB = blk.tile([P, TW], F16, tag="selB")
            seltB = blk.tile([P, TW], F16, tag="seltB")
            nc.sync.dma_start(selB[:], sel_d[:, j * TW:(j + 1) * TW])
            nc.sync.dma_start(seltB[:], selt_d[:, j * TW:(j + 1) * TW])
            xrj = xr_sb[:, j * C:(j + 1) * C]

            po = psacc.tile([P, 2 + C], F32, space="PSUM", tag="po")
            for t0 in range(0, tpb, 2):
                wid = 2 if t0 + 1 < tpb else 1
                WC = wid * C
                xlg = l2g.tile([P, 2 * C], F16, tag="xlg")
                pp = psum.tile([P, 2 * C], F32, space="PSUM", tag="ppx")
                for u in range(wid):
                    t = t0 + u
                    col = j * tpb + t
                    sl = slice(u * C, (u + 1) * C)
                    nc.gpsimd.indirect_dma_start(
                        out=xlg[:, sl], out_offset=None, in_=xl2t_d[:],
                        in_offset=bass.IndirectOffsetOnAxis(
                            ap=gidx[:, col:col + 1], axis=0))
                    nc.tensor.matmul(pp[:, sl], seltB[:, t * P:(t + 1) * P],
                                     xrj, start=True, stop=False)
                    nc.tensor.matmul(pp[:, sl], ident[:], xlg[:, sl],
                                     start=False, stop=True)
                lv = work.tile([P, 2 * C], F16, tag="lv")
                if (pcnt % 8) < DVEP8:
                    nc.vector.scalar_tensor_tensor(
                        lv[:, 0:WC], pp[:, 0:WC], NEG_SLOPE, pp[:, 0:WC],
                        op0=OP.mult, op1=OP.max)
                else:
                    nc.scalar.activation(lv[:, 0:WC], pp[:, 0:WC], AF.Prelu,
                                         alpha=NEG_SLOPE)
                pcnt += 1
                lm = work.tile([P, 2 * C], F16, tag="lm")
                nc.vector.tensor_tensor(
                    lm[:, 0:WC].rearrange("p (w c) -> p w c", w=wid),
                    lv[:, 0:WC].rearrange("p (w c) -> p w c", w=wid),
                    bass.AP(att2m[:].tensor, att2m[:].offset,
                            [att2m[:].ap[0], [0, wid], [1, C]]),
                    op=OP.mult)
                lg = small.tile([P, 2], F32, tag="lg2")
                nc.vector.reduce_sum(
                    lg[:, 0:wid].rearrange("p (w o) -> p w o", w=wid),
                    lm[:, 0:WC].rearrange("p (w o c) -> p w o c", w=wid, o=1),
                    axis=mybir.AxisListType.X)
                for u in range(wid):
                    t = t0 + u
                    sl = slice(u * C, (u + 1) * C)
                    wwx = work.tile([P, 2 + C], F16, tag="wwx2")
                    nc.scalar.activation(wwx[:, 0:2],
                                         bcast_cols(lg[:, u:u + 1], 2),
                                         AF.Exp)
                    if (scnt % 8) < SC8:
                        nc.vector.tensor_tensor(wwx[:, 2:2 + C], pp[:, sl],
                                                bcast_cols(wwx[:, 0:1], C),
                                                op=OP.mult)
                    else:
                        wcol = small.tile([P, 1], F32, tag="wcol")
                        nc.scalar.activation(wcol[:], lg[:, u:u + 1], AF.Exp)
                        nc.scalar.activation(wwx[:, 2:2 + C], pp[:, sl],
                                             AF.Copy, scale=wcol[:, 0:1])
                    scnt += 1
                    nc.tensor.matmul(po[:], selB[:, t * P:(t + 1) * P], wwx[:],
                                     start=(t == 0), stop=(t == tpb - 1))

            dn = small.tile([P, 1], F32, tag="dn2")
            nc.vector.tensor_scalar(dn[:], po[:, 0:1], 1e-30, None, op0=OP.add)
            rd = small.tile([P, 1], F32, tag="rd2")
            nc.vector.reciprocal(rd[:], dn[:])
            ob = gwork.tile([P, C], F32, tag="ob")
            nc.vector.tensor_scalar(ob[:], po[:, 2:2 + C], rd[:, 0:1], None,
                                    op0=OP.mult)
            oo = gwork.tile([P, C], F32, tag="oo")
            nc.vector.tensor_tensor(oo[:], ob[:], xrj, op=OP.subtract)
            nc.vector.tensor_tensor(oo[:], oo[:], b2m[:], op=OP.add)
            nc.sync.dma_start(out_d[j * P:(j + 1) * P, :], oo[:])

    nc.compile()
    return nc


# ---------------------------------------------------------------------------
# Entry point
# ---------------------------------------------------------------------------

_CACHE = {}


def kernel_ex(inputs, trace=False, trace_cores=None):
    x = np.asarray(inputs["x"], dtype=np.float32)
    edge_index = np.asarray(inputs["edge_index"], dtype=np.int32)
    N = x.shape[0]
    n_cores = 8

    cfg, per_core, newid = preprocess(x, edge_index, n_cores)
    packed = pack_weights(inputs, cfg)

    key = (N, x.shape[1], cfg["npad"], cfg["tpb"])
    if key not in _CACHE:
        _CACHE[key] = build_program(cfg)
    nc = _CACHE[key]

    nown = cfg["nbpc"] * P
    in_maps = []
    for c in range(n_cores):
        m = dict(
            xT=per_core[c]["xT"],
            w1=packed["w1"], w2=packed["w2"], w2bias=packed["w2bias"],
            att1m=packed["att1m"], att2m=packed["att2m"],
            b1m=packed["b1m"], b2m=packed["b2m"],
            gidx=per_core[c]["gidx"],
            sel=per_core[c]["sel"],
            selt=per_core[c]["selt"],
            xeT=per_core[c]["xeT"],
            ones=packed["ones"],
            ident=packed["ident"],
        )
        in_maps.append(m)

    kw = {}
    if trace:
        kw.update(trace=True, trace_cores=trace_cores or [0])
    res = run_bass_kernel_spmd(nc, in_maps, core_ids=list(range(n_cores)), **kw)
    full = np.concatenate([res.results[c]["out"] for c in range(n_cores)],
                          axis=0)
    out = full[newid].astype(np.float32)
    return out, res


def kernel(**inputs):
    return kernel_ex(inputs)[0]
